# revision 21
# baseline (speedup 1.0000x reference)
"""ConvLinformer self-attention on 8 Trainium2 NeuronCores.

Sharding: 8 cores = (batch b, sequence-half s); B=4, N=4096 -> each core owns
2048 sequence rows of one batch. The conv (the dominant 275 GFLOP op) contracts
over the sequence dim, so each core computes a partial conv over its own rows
using only its half of the conv weight (host pre-transposed to [i, k, o] layout
for clean DMA + matmul lhsT tiles); a pairwise AllReduce of the small (256,1024)
conv output completes it. Attention (all 8 heads, own rows) then needs no
further communication, and neither does the output projection.

Changes vs the 885us baseline (final: ~694us, rel err ~1.3e-2 vs 2e-2 gate):
- All matmul inputs in bf16 (PSUM accumulation stays fp32): streams as fast
  as fp32r at 512-wide free dims, enables the compiler's automatic
  fast-weight-load on every stationary operand, halves weight/x DMA traffic
  (92 -> 52 MB/core), and keeps the chip out of the power-management
  downclock state that fp32r versions trigger stochastically (fp32r runs
  oscillated 756-899us run-to-run; bf16 runs sit stably at ~694).
- xT DMA'd in 16 i-major blocks (first keys matmul starts ~7us in, not
  ~30us); Wq and xT tail blocks trickle in via the idle GpSimd DMA queue so
  startup-critical transfers own the bandwidth.
- The conv-output AllReduce (bf16, pairwise) is issued BEFORE the q
  projection, hiding its ~25us trigger+transfer under ~55us of q matmuls and
  keeping the PE busy through the transition (no HAM clock-gate re-throttle).
- Attention: softmax denominators come from a ones[128,128]-lhsT matmul that
  broadcasts the K-sum to all 128 partitions at the same PE cost as a 1-row
  output (matmul cost is output-free-size only); each iteration then needs
  only reciprocal_approx_fast + tensor_mul on Vector, nothing on the PE
  critical path. (The baseline's per-iteration [1,512] nc.vector.reciprocal
  at 3.3us each stalled the PSUM ring and cost ~110us.)
- Keys/q loops ordered contraction-chunk-outer so each stationary weight
  load feeds 2-4 matmuls. Matmul free dims stay at 512: walrus rejects
  1024-free/2-PSUM-bank outputs (NCC_IXCG864).
"""

import sys

sys.path.insert(0, "/opt/trn_rl_repo")

import numpy as np

B, N, D = 4, 4096, 1024
H, DH, K = 8, 128, 256
KER = 32
PADL = 15
NL = N // 2          # rows per core
NCORES = 8
SCALE = DH ** -0.5

_CACHE = {}


def _build(single_core=False, phases=5):
    import concourse.bacc as bacc
    import concourse.mybir as mybir
    import concourse.tile as tile

    FP32 = mybir.dt.float32
    FP32R = mybir.dt.float32r
    BF16 = mybir.dt.bfloat16
    ACTF = mybir.ActivationFunctionType

    nc = bacc.Bacc("TRN2", target_bir_lowering=False, debug=False,
                   num_devices=1 if single_core else NCORES)

    IT = NL // 128        # 16 i-tiles
    TPAD = 1056           # padded conv spatial width (15 + 1024 + 17)

    xTim = nc.dram_tensor("xTim", (IT, 128, 8, 128), BF16, kind="ExternalInput")
    wqT = nc.dram_tensor("wqT", (D, D), BF16, kind="ExternalInput")
    wkT = nc.dram_tensor("wkT", (D, D), BF16, kind="ExternalInput")
    woT = nc.dram_tensor("woT", (D, D), BF16, kind="ExternalInput")
    wpkT = nc.dram_tensor("wpkT", (NL, KER * K), BF16, kind="ExternalInput")
    bpk_in = nc.dram_tensor("bpk", (128, 2), FP32, kind="ExternalInput")
    bo_in = nc.dram_tensor("bo", (1, D), FP32, kind="ExternalInput")
    ones_in = nc.dram_tensor("ones", (128, 128), BF16, kind="ExternalInput")
    ident_in = nc.dram_tensor("ident", (128, 128), BF16, kind="ExternalInput")
    y_out = nc.dram_tensor("y", (NL, D), FP32, kind="ExternalOutput")

    with tile.TileContext(nc) as tc:
            # ---- long-lived pools -------------------------------------------
            consts = tc.alloc_tile_pool(name="consts", bufs=1, side="left")
            p_wq = tc.alloc_tile_pool(name="wq", bufs=1, side="left")
            p_x = tc.alloc_tile_pool(name="x", bufs=1, side="left")
            p_wk = tc.alloc_tile_pool(name="wk", bufs=1, side="left")
            p_slab = tc.alloc_tile_pool(name="slab", bufs=4, side="left")
            p_keys = tc.alloc_tile_pool(name="keys", bufs=3, side="left")
            ps_conv = tc.alloc_tile_pool(name="convps", bufs=1, space="PSUM")
            ps_k = tc.alloc_tile_pool(name="kps", bufs=4, space="PSUM")

            ones = consts.tile([128, 128], BF16, tag="ones")
            bpk_t = consts.tile([128, 2], FP32, tag="bpk")
            bo_row = consts.tile([1, D], FP32, tag="borow")
            bo_bc = consts.tile([128, D], FP32, tag="bobc")

            xT_s = p_x.tile([128, 8, NL], BF16, tag="xT")
            wkT_s = p_wk.tile([128, 8, D], BF16, tag="wk")
            wqT_s = p_wq.tile([128, 8, D], BF16, tag="wq")
            # startup-critical DMA order: wk chunks, first x block, first conv
            # slab, then the rest (wq can trickle in under the conv phase).
            nc.sync.dma_start(out=xT_s[:, :, 0:128], in_=xTim.ap()[0])
            nc.sync.dma_start(out=wkT_s[:, 0, :], in_=wkT.ap()[0:128, :])

            slabs = {}

            def slab_dma(i, kh):
                t = p_slab.tile([128, 16 * K], BF16, tag="slab")
                nc.sync.dma_start(
                    out=t[:],
                    in_=wpkT.ap()[i * 128:(i + 1) * 128,
                                  kh * 16 * K:(kh + 1) * 16 * K])
                slabs[(i, kh)] = t

            slab_dma(0, 0)
            for a in range(1, 8):
                nc.sync.dma_start(out=wkT_s[:, a, :],
                                  in_=wkT.ap()[a * 128:(a + 1) * 128, :])
            nc.sync.dma_start(out=xT_s[:, :, 128:256], in_=xTim.ap()[1])
            slab_dma(0, 1)
            nc.gpsimd.dma_start(out=ones[:], in_=ones_in.ap())
            nc.gpsimd.dma_start(out=bpk_t[:], in_=bpk_in.ap())
            nc.gpsimd.dma_start(out=bo_row[:], in_=bo_in.ap())
            nc.gpsimd.partition_broadcast(bo_bc[:], bo_row[:])

            def deferred_dma(i):
                # trickle in xT block i+2 and one wq chunk per conv i-step so
                # startup-critical transfers get the full DMA bandwidth
                if i + 2 < IT:
                    nc.gpsimd.dma_start(
                        out=xT_s[:, :, (i + 2) * 128:(i + 3) * 128],
                        in_=xTim.ap()[i + 2])
                if 0 <= i - 1 < 8:
                    a = i - 1
                    nc.gpsimd.dma_start(out=wqT_s[:, a, :],
                                        in_=wqT.ap()[a * 128:(a + 1) * 128, :])

            # ---- P1: keys production + conv accumulation --------------------
            cps = [[ps_conv.tile([128, 512], FP32, tag=f"cps{o}{t}", name=f"cps{o}{t}")
                    for t in range(2)] for o in range(2)]

            def emit_keys(i):
                pks = [ps_k.tile([128, 512], FP32, tag="psk",
                                 name=f"psk{i}_{t}") for t in range(2)]
                for a in range(8):
                    for tch in range(2):
                        nc.tensor.matmul(
                            pks[tch][:], xT_s[:, a, i * 128:(i + 1) * 128],
                            wkT_s[:, a, tch * 512:(tch + 1) * 512],
                            start=(a == 0), stop=(a == 7))
                kt = p_keys.tile([128, TPAD], BF16, tag="keys")
                # fp32r memset is invalid ISA; Copy(in*0.0) writes fp32r zeros
                nc.scalar.activation(kt[:, 0:PADL], pks[0][:, 0:PADL],
                                     ACTF.Copy, scale=0.0)
                nc.scalar.activation(kt[:, PADL + D:TPAD],
                                     pks[1][:, 0:TPAD - PADL - D],
                                     ACTF.Copy, scale=0.0)
                nc.scalar.activation(kt[:, PADL:PADL + 512], pks[0][:], ACTF.Copy)
                nc.scalar.activation(kt[:, PADL + 512:PADL + D], pks[1][:], ACTF.Copy)
                return kt

            kt_cur = emit_keys(0)
            for i in range(IT):
                deferred_dma(i)
                kt_next = emit_keys(i + 1) if i + 1 < IT else None
                for kh in range(2):
                    # prefetch the slab two (i,kh)-steps ahead
                    nxt = (i, kh + 1) if kh == 0 else (i + 1, 0)
                    nxt2 = (nxt[0], 1) if nxt[1] == 0 else (nxt[0] + 1, 0)
                    if nxt2[0] < IT and nxt2 not in slabs:
                        slab_dma(*nxt2)
                    slab = slabs.pop((i, kh))
                    for k16 in range(16):
                        k = kh * 16 + k16
                        for och in range(2):
                            lhsT = slab[:, k16 * K + och * 128:k16 * K + och * 128 + 128]
                            for tch in range(2):
                                nc.tensor.matmul(
                                    cps[och][tch][:], lhsT,
                                    kt_cur[:, k + tch * 512:k + tch * 512 + 512],
                                    start=(i == 0 and k == 0),
                                    stop=(i == IT - 1 and k == KER - 1))
                kt_cur = kt_next

            p_keys.release()
            p_slab.release()
            p_wk.release()
            ps_k.release()

            # ---- P2a: issue the conv AllReduce (overlaps q projection) ------
            p_kc = tc.alloc_tile_pool(name="kc", bufs=1, side="right")
            p_dram = tc.alloc_tile_pool(name="cc", bufs=1, space="DRAM")
            cc_in = p_dram.tile([2, 128, D], BF16, tag="ccin")
            cc_out = p_dram.tile([2, 128, D], BF16, tag="ccout")
            for och in range(2):
                kcp = p_kc.tile([128, D], BF16, tag="kcio")
                for tch in range(2):
                    nc.scalar.activation(kcp[:, tch * 512:(tch + 1) * 512],
                                         cps[och][tch][:], ACTF.Copy)
                nc.sync.dma_start(out=cc_in[och], in_=kcp[:])
            if single_core:
                nc.sync.dma_start(out=cc_out[:], in_=cc_in[:])
            else:
                nc.gpsimd.collective_compute(
                    "AllReduce", mybir.AluOpType.add,
                    replica_groups=[[0, 1], [2, 3], [4, 5], [6, 7]],
                    ins=[cc_in[:]], outs=[cc_out[:]])
            ps_conv.release()

            # ---- P2b: qT = Wq @ x^T  (t on partitions, n free) --------------
            assert phases >= 2
            p_qT = tc.alloc_tile_pool(name="qT", bufs=1, side="right")
            qT_s = p_qT.tile([128, 8, NL], BF16, tag="qT")
            ps_q = tc.alloc_tile_pool(name="qps", bufs=8, space="PSUM")
            for tt in range(8):
                psqs = [ps_q.tile([128, 512], FP32, tag="psq",
                                  name=f"psq{tt}_{n}") for n in range(4)]
                for a in range(8):
                    for nch in range(4):
                        nc.tensor.matmul(
                            psqs[nch][:], wqT_s[:, a, tt * 128:(tt + 1) * 128],
                            xT_s[:, a, nch * 512:(nch + 1) * 512],
                            start=(a == 0), stop=(a == 7))
                for nch in range(4):
                    nc.scalar.activation(qT_s[:, tt, nch * 512:(nch + 1) * 512],
                                         psqs[nch][:], ACTF.Copy)
            ps_q.release()
            p_x.release()
            p_wq.release()
            p_wo = tc.alloc_tile_pool(name="wo", bufs=1, side="left")
            woT_s = p_wo.tile([128, 8, D], BF16, tag="wo")
            for a in range(8):
                nc.sync.dma_start(out=woT_s[:, a, :],
                                  in_=woT.ap()[a * 128:(a + 1) * 128, :])

            # ---- P3: finish conv: +bias, transpose --------------------------
            p_kcf = tc.alloc_tile_pool(name="kcf", bufs=1, side="right")
            ps_t = tc.alloc_tile_pool(name="tps", bufs=2, space="PSUM")

            kc_b = p_kcf.tile([128, 2, D], BF16, tag="kcb")
            for och in range(2):
                kcs = p_kc.tile([128, D], BF16, tag="kcio")
                nc.sync.dma_start(out=kcs[:], in_=cc_out[och])
                nc.vector.tensor_scalar_add(kc_b[:, och, :], kcs[:],
                                            bpk_t[:, och:och + 1])
            kcT = p_kcf.tile([128, 8, K], BF16, tag="kcT")
            ident = p_kcf.tile([128, 128], BF16, tag="ident")
            nc.gpsimd.dma_start(out=ident[:], in_=ident_in.ap())
            for tt in range(8):
                pst = ps_t.tile([128, K], BF16, tag="pst")
                nc.tensor.transpose(pst[:, 0:128],
                                    kc_b[:, 0, tt * 128:(tt + 1) * 128], ident[:])
                nc.tensor.transpose(pst[:, 128:256],
                                    kc_b[:, 1, tt * 128:(tt + 1) * 128], ident[:])
                nc.scalar.activation(kcT[:, tt, :], pst[:], ACTF.Copy)
            ps_t.release()

            # ---- P4: attention (unnormalized; denominators batched) ---------
            assert phases >= 4
            p_attn = tc.alloc_tile_pool(name="attnT", bufs=1, side="left")
            p_exp = tc.alloc_tile_pool(name="exp", bufs=6, side="right")
            p_recb = tc.alloc_tile_pool(name="recb", bufs=3, side="right")
            ps_d = tc.alloc_tile_pool(name="dps", bufs=4, space="PSUM")
            ps_s = tc.alloc_tile_pool(name="sps", bufs=2, space="PSUM")
            ps_a = tc.alloc_tile_pool(name="aps", bufs=2, space="PSUM")

            attn_outT = p_attn.tile([128, 8, NL], BF16, tag="attnT")

            def emit_dots(nch, h):
                nsl = slice(nch * 512, (nch + 1) * 512)
                psd_lo = ps_d.tile([128, 512], FP32, tag="psd", name=f"psdl{nch}_{h}")
                nc.tensor.matmul(psd_lo[:], kcT[:, h, 0:128], qT_s[:, h, nsl],
                                 start=True, stop=True)
                psd_hi = ps_d.tile([128, 512], FP32, tag="psd", name=f"psdh{nch}_{h}")
                nc.tensor.matmul(psd_hi[:], kcT[:, h, 128:256], qT_s[:, h, nsl],
                                 start=True, stop=True)
                e_lo = p_exp.tile([128, 512], BF16, tag="exp", name=f"el{nch}_{h}")
                nc.scalar.activation(e_lo[:], psd_lo[:], ACTF.Exp, scale=SCALE)
                e_hi = p_exp.tile([128, 512], BF16, tag="exp", name=f"eh{nch}_{h}")
                nc.scalar.activation(e_hi[:], psd_hi[:], ACTF.Exp, scale=SCALE)
                return nsl, e_lo, e_hi

            def emit_tail(nch, h, st):
                nsl, e_lo, e_hi = st
                # denominator: ones lhsT broadcasts the K-sum to all 128
                # partitions at the same PE cost as a [1,512] output
                pss = ps_s.tile([128, 512], FP32, tag="pss", name=f"pss{nch}_{h}")
                nc.tensor.matmul(pss[:], ones[:], e_lo[:],
                                 start=True, stop=False)
                nc.tensor.matmul(pss[:], ones[:], e_hi[:],
                                 start=False, stop=True)
                psa = ps_a.tile([128, 512], FP32, tag="psa", name=f"psa{nch}_{h}")
                nc.tensor.matmul(psa[:], kc_b[:, 0, h * 128:(h + 1) * 128],
                                 e_lo[:], start=True, stop=False)
                nc.tensor.matmul(psa[:], kc_b[:, 1, h * 128:(h + 1) * 128],
                                 e_hi[:], start=False, stop=True)
                rec = p_recb.tile([128, 512], FP32, tag="rec",
                                  name=f"rec{nch}_{h}")
                nc.vector.reciprocal_approx_fast(rec[:], pss[:])
                nc.vector.tensor_mul(attn_outT[:, h, nsl], psa[:], rec[:])

            seq = [(nch, h) for nch in range(4) for h in range(8)]
            st = emit_dots(*seq[0])
            for j, (nch, h) in enumerate(seq):
                nxt = emit_dots(*seq[j + 1]) if j + 1 < len(seq) else None
                emit_tail(nch, h, st)
                st = nxt

            p_recb.release()
            p_exp.release()
            p_kcf.release()
            p_qT.release()
            p_kc.release()
            ps_a.release()
            ps_s.release()
            ps_d.release()

            # ---- P5: normalize + y = attn_out @ Wo^T + bo, interleaved ------
            assert phases >= 5
            p_y = tc.alloc_tile_pool(name="ysb", bufs=3, side="right")
            ps_y = tc.alloc_tile_pool(name="yps", bufs=4, space="PSUM")
            for nt in range(16):
                psy = [ps_y.tile([128, 512], FP32, tag="psy",
                                 name=f"psy{nt}_{_i}") for _i in range(2)]
                for tt in range(8):
                    for cch in range(2):
                        nc.tensor.matmul(
                            psy[cch][:],
                            attn_outT[:, tt, nt * 128:(nt + 1) * 128],
                            woT_s[:, tt, cch * 512:(cch + 1) * 512],
                            start=(tt == 0), stop=(tt == 7))
                ysb = p_y.tile([128, D], FP32, tag="ysb")
                for cch in range(2):
                    nc.vector.tensor_add(ysb[:, cch * 512:(cch + 1) * 512],
                                         psy[cch][:],
                                         bo_bc[:, cch * 512:(cch + 1) * 512])
                nc.sync.dma_start(out=y_out.ap()[nt * 128:(nt + 1) * 128, :],
                                  in_=ysb[:])
            p_y.release()
            ps_y.release()
            p_attn.release()
            p_wo.release()
            consts.release()

    nc.compile()
    return nc


def _get_nc():
    if "nc" not in _CACHE:
        _CACHE["nc"] = _build()
    return _CACHE["nc"]


def _prep_inputs(x, Wq, Wk, Wpk, bpk, Wo, bo):
    import ml_dtypes
    bf = ml_dtypes.bfloat16
    wqT = np.ascontiguousarray(Wq.T.astype(bf))
    wkT = np.ascontiguousarray(Wk.T.astype(bf))
    woT = np.ascontiguousarray(Wo.T.astype(bf))
    # Wpk (K=256, N=4096, KER=32) -> [i, k, o] contiguous
    wpkT = np.ascontiguousarray(
        Wpk.astype(bf).transpose(1, 2, 0)).reshape(N, KER * K)
    bpk2 = np.ascontiguousarray(bpk.astype(np.float32).reshape(2, 128).T)
    bo2 = np.ascontiguousarray(bo.astype(np.float32).reshape(1, D))
    ones = np.ones((128, 128), dtype=bf)
    ident = np.eye(128, dtype=bf)
    in_maps = []
    for c in range(NCORES):
        b, s = c // 2, c % 2
        xT = x[b, s * NL:(s + 1) * NL, :].T.astype(bf)  # (D, NL)
        xTim = np.ascontiguousarray(
            xT.reshape(8, 128, NL // 128, 128).transpose(2, 1, 0, 3))
        in_maps.append({
            "xTim": xTim,
            "wqT": wqT, "wkT": wkT, "woT": woT,
            "wpkT": np.ascontiguousarray(wpkT[s * NL:(s + 1) * NL]),
            "bpk": bpk2, "bo": bo2, "ones": ones, "ident": ident,
        })
    return in_maps


def kernel(x, Wq, Wk, Wpk, bpk, Wo, bo, _trace=False, _trace_kwargs=None):
    from concourse.bass_utils import run_bass_kernel_spmd

    nc = _get_nc()
    in_maps = _prep_inputs(np.asarray(x), np.asarray(Wq), np.asarray(Wk),
                           np.asarray(Wpk), np.asarray(bpk), np.asarray(Wo),
                           np.asarray(bo))
    res = run_bass_kernel_spmd(nc, in_maps, core_ids=list(range(NCORES)),
                               trace=_trace, **(_trace_kwargs or {}))
    _CACHE["last_result"] = res
    out = np.empty((B, N, D), dtype=np.float32)
    for c in range(NCORES):
        b, s = c // 2, c % 2
        out[b, s * NL:(s + 1) * NL, :] = res.results[c]["y"]
    return out


# revision 22
# speedup vs baseline: 1.0045x; 1.0045x over previous
"""ConvLinformer self-attention on 8 Trainium2 NeuronCores.

Sharding: 8 cores = (batch b, sequence-half s); B=4, N=4096 -> each core owns
2048 sequence rows of one batch. The conv (the dominant 275 GFLOP op) contracts
over the sequence dim, so each core computes a partial conv over its own rows
using only its half of the conv weight (host pre-transposed to [i, k, o] layout
for clean DMA + matmul lhsT tiles); a pairwise AllReduce of the small (256,1024)
conv output completes it. Attention (all 8 heads, own rows) then needs no
further communication, and neither does the output projection.

Changes vs the 885us baseline (final: ~694us, rel err ~1.3e-2 vs 2e-2 gate):
- All matmul inputs in bf16 (PSUM accumulation stays fp32): streams as fast
  as fp32r at 512-wide free dims, enables the compiler's automatic
  fast-weight-load on every stationary operand, halves weight/x DMA traffic
  (92 -> 52 MB/core), and keeps the chip out of the power-management
  downclock state that fp32r versions trigger stochastically (fp32r runs
  oscillated 756-899us run-to-run; bf16 runs sit stably at ~694).
- xT DMA'd in 16 i-major blocks (first keys matmul starts ~7us in, not
  ~30us); Wq and xT tail blocks trickle in via the idle GpSimd DMA queue so
  startup-critical transfers own the bandwidth.
- The conv-output AllReduce (bf16, pairwise) is issued BEFORE the q
  projection, hiding its ~25us trigger+transfer under ~55us of q matmuls and
  keeping the PE busy through the transition (no HAM clock-gate re-throttle).
- Attention: softmax denominators come from a ones[128,128]-lhsT matmul that
  broadcasts the K-sum to all 128 partitions at the same PE cost as a 1-row
  output (matmul cost is output-free-size only); each iteration then needs
  only reciprocal_approx_fast + tensor_mul on Vector, nothing on the PE
  critical path. (The baseline's per-iteration [1,512] nc.vector.reciprocal
  at 3.3us each stalled the PSUM ring and cost ~110us.)
- Keys/q loops ordered contraction-chunk-outer so each stationary weight
  load feeds 2-4 matmuls. Matmul free dims stay at 512: walrus rejects
  1024-free/2-PSUM-bank outputs (NCC_IXCG864).
"""

import sys

sys.path.insert(0, "/opt/trn_rl_repo")

import numpy as np

B, N, D = 4, 4096, 1024
H, DH, K = 8, 128, 256
KER = 32
PADL = 15
NL = N // 2          # rows per core
NCORES = 8
SCALE = DH ** -0.5

_CACHE = {}


def _build(single_core=False, phases=5):
    import concourse.bacc as bacc
    import concourse.mybir as mybir
    import concourse.tile as tile

    FP32 = mybir.dt.float32
    FP32R = mybir.dt.float32r
    BF16 = mybir.dt.bfloat16
    ACTF = mybir.ActivationFunctionType

    nc = bacc.Bacc("TRN2", target_bir_lowering=False, debug=False,
                   num_devices=1 if single_core else NCORES)

    IT = NL // 128        # 16 i-tiles
    TPAD = 1056           # padded conv spatial width (15 + 1024 + 17)

    xTim = nc.dram_tensor("xTim", (IT, 128, 8, 128), BF16, kind="ExternalInput")
    wqT = nc.dram_tensor("wqT", (D, D), BF16, kind="ExternalInput")
    wkT = nc.dram_tensor("wkT", (D, D), BF16, kind="ExternalInput")
    woT = nc.dram_tensor("woT", (D, D), BF16, kind="ExternalInput")
    wpkT = nc.dram_tensor("wpkT", (NL, KER * K), BF16, kind="ExternalInput")
    bpk_in = nc.dram_tensor("bpk", (128, 2), FP32, kind="ExternalInput")
    bo_in = nc.dram_tensor("bo", (1, D), FP32, kind="ExternalInput")
    ones_in = nc.dram_tensor("ones", (128, 128), BF16, kind="ExternalInput")
    ident_in = nc.dram_tensor("ident", (128, 128), BF16, kind="ExternalInput")
    y_out = nc.dram_tensor("y", (NL, D), FP32, kind="ExternalOutput")

    with tile.TileContext(nc) as tc:
            # ---- long-lived pools -------------------------------------------
            consts = tc.alloc_tile_pool(name="consts", bufs=1, side="left")
            p_wq = tc.alloc_tile_pool(name="wq", bufs=1, side="left")
            p_x = tc.alloc_tile_pool(name="x", bufs=1, side="left")
            p_wk = tc.alloc_tile_pool(name="wk", bufs=1, side="left")
            p_slab = tc.alloc_tile_pool(name="slab", bufs=4, side="left")
            p_keys = tc.alloc_tile_pool(name="keys", bufs=3, side="left")
            ps_conv = tc.alloc_tile_pool(name="convps", bufs=1, space="PSUM")
            ps_k = tc.alloc_tile_pool(name="kps", bufs=4, space="PSUM")

            ones = consts.tile([128, 128], BF16, tag="ones")
            bpk_t = consts.tile([128, 2], FP32, tag="bpk")
            bo_row = consts.tile([1, D], FP32, tag="borow")
            bo_bc = consts.tile([128, D], FP32, tag="bobc")

            xT_s = p_x.tile([128, 8, NL], BF16, tag="xT")
            wkT_s = p_wk.tile([128, 8, D], BF16, tag="wk")
            wqT_s = p_wq.tile([128, 8, D], BF16, tag="wq")
            # startup-critical DMA order: wk chunks, first x block, first conv
            # slab, then the rest (wq can trickle in under the conv phase).
            nc.sync.dma_start(out=xT_s[:, :, 0:128], in_=xTim.ap()[0])
            nc.sync.dma_start(out=wkT_s[:, 0, :], in_=wkT.ap()[0:128, :])

            slabs = {}

            def slab_dma(i, kh):
                t = p_slab.tile([128, 16 * K], BF16, tag="slab")
                nc.sync.dma_start(
                    out=t[:],
                    in_=wpkT.ap()[i * 128:(i + 1) * 128,
                                  kh * 16 * K:(kh + 1) * 16 * K])
                slabs[(i, kh)] = t

            slab_dma(0, 0)
            for a in range(1, 8):
                nc.sync.dma_start(out=wkT_s[:, a, :],
                                  in_=wkT.ap()[a * 128:(a + 1) * 128, :])
            nc.sync.dma_start(out=xT_s[:, :, 128:256], in_=xTim.ap()[1])
            slab_dma(0, 1)
            nc.gpsimd.dma_start(out=ones[:], in_=ones_in.ap())
            nc.gpsimd.dma_start(out=bpk_t[:], in_=bpk_in.ap())
            nc.gpsimd.dma_start(out=bo_row[:], in_=bo_in.ap())
            nc.gpsimd.partition_broadcast(bo_bc[:], bo_row[:])

            def deferred_dma(i):
                # trickle in xT block i+2 and one wq chunk per conv i-step so
                # startup-critical transfers get the full DMA bandwidth
                if i + 2 < IT:
                    nc.gpsimd.dma_start(
                        out=xT_s[:, :, (i + 2) * 128:(i + 3) * 128],
                        in_=xTim.ap()[i + 2])
                if 0 <= i - 1 < 8:
                    a = i - 1
                    nc.gpsimd.dma_start(out=wqT_s[:, a, :],
                                        in_=wqT.ap()[a * 128:(a + 1) * 128, :])

            # ---- P1: keys production + conv accumulation --------------------
            cps = [[ps_conv.tile([128, 512], FP32, tag=f"cps{o}{t}", name=f"cps{o}{t}")
                    for t in range(2)] for o in range(2)]

            def emit_keys(i):
                pks = [ps_k.tile([128, 512], FP32, tag="psk",
                                 name=f"psk{i}_{t}") for t in range(2)]
                for a in range(8):
                    for tch in range(2):
                        nc.tensor.matmul(
                            pks[tch][:], xT_s[:, a, i * 128:(i + 1) * 128],
                            wkT_s[:, a, tch * 512:(tch + 1) * 512],
                            start=(a == 0), stop=(a == 7))
                kt = p_keys.tile([128, TPAD], BF16, tag="keys")
                # fp32r memset is invalid ISA; Copy(in*0.0) writes fp32r zeros
                nc.scalar.activation(kt[:, 0:PADL], pks[0][:, 0:PADL],
                                     ACTF.Copy, scale=0.0)
                nc.scalar.activation(kt[:, PADL + D:TPAD],
                                     pks[1][:, 0:TPAD - PADL - D],
                                     ACTF.Copy, scale=0.0)
                nc.scalar.activation(kt[:, PADL:PADL + 512], pks[0][:], ACTF.Copy)
                nc.scalar.activation(kt[:, PADL + 512:PADL + D], pks[1][:], ACTF.Copy)
                return kt

            kt_cur = emit_keys(0)
            for i in range(IT):
                deferred_dma(i)
                kt_next = emit_keys(i + 1) if i + 1 < IT else None
                for kh in range(2):
                    # prefetch the slab two (i,kh)-steps ahead
                    nxt = (i, kh + 1) if kh == 0 else (i + 1, 0)
                    nxt2 = (nxt[0], 1) if nxt[1] == 0 else (nxt[0] + 1, 0)
                    if nxt2[0] < IT and nxt2 not in slabs:
                        slab_dma(*nxt2)
                    slab = slabs.pop((i, kh))
                    for k16 in range(16):
                        k = kh * 16 + k16
                        for och in range(2):
                            lhsT = slab[:, k16 * K + och * 128:k16 * K + och * 128 + 128]
                            for tch in range(2):
                                nc.tensor.matmul(
                                    cps[och][tch][:], lhsT,
                                    kt_cur[:, k + tch * 512:k + tch * 512 + 512],
                                    start=(i == 0 and k == 0),
                                    stop=(i == IT - 1 and k == KER - 1))
                kt_cur = kt_next

            p_keys.release()
            p_slab.release()
            p_wk.release()
            ps_k.release()

            # ---- P2a: issue the conv AllReduce (overlaps q projection) ------
            p_kc = tc.alloc_tile_pool(name="kc", bufs=1, side="right")
            p_dram = tc.alloc_tile_pool(name="cc", bufs=1, space="DRAM")
            cc_in = p_dram.tile([2, 128, D], BF16, tag="ccin")
            cc_out = p_dram.tile([2, 128, D], BF16, tag="ccout")
            for och in range(2):
                kcp = p_kc.tile([128, D], BF16, tag="kcio")
                for tch in range(2):
                    nc.scalar.activation(kcp[:, tch * 512:(tch + 1) * 512],
                                         cps[och][tch][:], ACTF.Copy)
                nc.sync.dma_start(out=cc_in[och], in_=kcp[:])
            if single_core:
                nc.sync.dma_start(out=cc_out[:], in_=cc_in[:])
            else:
                nc.gpsimd.collective_compute(
                    "AllReduce", mybir.AluOpType.add,
                    replica_groups=[[0, 1], [2, 3], [4, 5], [6, 7]],
                    ins=[cc_in[:]], outs=[cc_out[:]])

            # ---- P2b: qT = Wq @ x^T  (t on partitions, n free) --------------
            assert phases >= 2
            p_qT = tc.alloc_tile_pool(name="qT", bufs=1, side="right")
            qT_s = p_qT.tile([128, 8, NL], BF16, tag="qT")
            ps_q = tc.alloc_tile_pool(name="qps", bufs=4, space="PSUM")
            for tt in range(8):
                psqs = [ps_q.tile([128, 512], FP32, tag="psq",
                                  name=f"psq{tt}_{n}") for n in range(4)]
                for a in range(8):
                    for nch in range(4):
                        nc.tensor.matmul(
                            psqs[nch][:], wqT_s[:, a, tt * 128:(tt + 1) * 128],
                            xT_s[:, a, nch * 512:(nch + 1) * 512],
                            start=(a == 0), stop=(a == 7))
                for nch in range(4):
                    nc.scalar.activation(qT_s[:, tt, nch * 512:(nch + 1) * 512],
                                         psqs[nch][:], ACTF.Copy)
            ps_q.release()
            ps_conv.release()
            p_x.release()
            p_wq.release()
            p_wo = tc.alloc_tile_pool(name="wo", bufs=1, side="left")
            woT_s = p_wo.tile([128, 8, D], BF16, tag="wo")
            for a in range(8):
                nc.sync.dma_start(out=woT_s[:, a, :],
                                  in_=woT.ap()[a * 128:(a + 1) * 128, :])

            # ---- P3: finish conv: +bias, transpose --------------------------
            p_kcf = tc.alloc_tile_pool(name="kcf", bufs=1, side="right")
            ps_t = tc.alloc_tile_pool(name="tps", bufs=2, space="PSUM")

            kc_b = p_kcf.tile([128, 2, D], BF16, tag="kcb")
            for och in range(2):
                kcs = p_kc.tile([128, D], BF16, tag="kcio")
                nc.sync.dma_start(out=kcs[:], in_=cc_out[och])
                nc.vector.tensor_scalar_add(kc_b[:, och, :], kcs[:],
                                            bpk_t[:, och:och + 1])
            kcT = p_kcf.tile([128, 8, K], BF16, tag="kcT")
            ident = p_kcf.tile([128, 128], BF16, tag="ident")
            nc.gpsimd.dma_start(out=ident[:], in_=ident_in.ap())
            for tt in range(8):
                pst = ps_t.tile([128, K], BF16, tag="pst")
                nc.tensor.transpose(pst[:, 0:128],
                                    kc_b[:, 0, tt * 128:(tt + 1) * 128], ident[:])
                nc.tensor.transpose(pst[:, 128:256],
                                    kc_b[:, 1, tt * 128:(tt + 1) * 128], ident[:])
                nc.scalar.activation(kcT[:, tt, :], pst[:], ACTF.Copy)
            ps_t.release()

            # ---- P4: attention (unnormalized; denominators batched) ---------
            assert phases >= 4
            p_attn = tc.alloc_tile_pool(name="attnT", bufs=1, side="left")
            p_exp = tc.alloc_tile_pool(name="exp", bufs=6, side="right")
            p_recb = tc.alloc_tile_pool(name="recb", bufs=3, side="right")
            ps_d = tc.alloc_tile_pool(name="dps", bufs=4, space="PSUM")
            ps_s = tc.alloc_tile_pool(name="sps", bufs=2, space="PSUM")
            ps_a = tc.alloc_tile_pool(name="aps", bufs=2, space="PSUM")

            attn_outT = p_attn.tile([128, 8, NL], BF16, tag="attnT")

            def emit_dots(nch, h):
                nsl = slice(nch * 512, (nch + 1) * 512)
                psd_lo = ps_d.tile([128, 512], FP32, tag="psd", name=f"psdl{nch}_{h}")
                nc.tensor.matmul(psd_lo[:], kcT[:, h, 0:128], qT_s[:, h, nsl],
                                 start=True, stop=True)
                psd_hi = ps_d.tile([128, 512], FP32, tag="psd", name=f"psdh{nch}_{h}")
                nc.tensor.matmul(psd_hi[:], kcT[:, h, 128:256], qT_s[:, h, nsl],
                                 start=True, stop=True)
                e_lo = p_exp.tile([128, 512], BF16, tag="exp", name=f"el{nch}_{h}")
                nc.scalar.activation(e_lo[:], psd_lo[:], ACTF.Exp, scale=SCALE)
                e_hi = p_exp.tile([128, 512], BF16, tag="exp", name=f"eh{nch}_{h}")
                nc.scalar.activation(e_hi[:], psd_hi[:], ACTF.Exp, scale=SCALE)
                return nsl, e_lo, e_hi

            def emit_tail(nch, h, st):
                nsl, e_lo, e_hi = st
                # denominator: ones lhsT broadcasts the K-sum to all 128
                # partitions at the same PE cost as a [1,512] output
                pss = ps_s.tile([128, 512], FP32, tag="pss", name=f"pss{nch}_{h}")
                nc.tensor.matmul(pss[:], ones[:], e_lo[:],
                                 start=True, stop=False)
                nc.tensor.matmul(pss[:], ones[:], e_hi[:],
                                 start=False, stop=True)
                psa = ps_a.tile([128, 512], FP32, tag="psa", name=f"psa{nch}_{h}")
                nc.tensor.matmul(psa[:], kc_b[:, 0, h * 128:(h + 1) * 128],
                                 e_lo[:], start=True, stop=False)
                nc.tensor.matmul(psa[:], kc_b[:, 1, h * 128:(h + 1) * 128],
                                 e_hi[:], start=False, stop=True)
                rec = p_recb.tile([128, 512], FP32, tag="rec",
                                  name=f"rec{nch}_{h}")
                nc.vector.reciprocal_approx_fast(rec[:], pss[:])
                nc.vector.tensor_mul(attn_outT[:, h, nsl], psa[:], rec[:])

            seq = [(nch, h) for nch in range(4) for h in range(8)]
            st = emit_dots(*seq[0])
            for j, (nch, h) in enumerate(seq):
                nxt = emit_dots(*seq[j + 1]) if j + 1 < len(seq) else None
                emit_tail(nch, h, st)
                st = nxt

            p_recb.release()
            p_exp.release()
            p_kcf.release()
            p_qT.release()
            p_kc.release()
            ps_a.release()
            ps_s.release()
            ps_d.release()

            # ---- P5: normalize + y = attn_out @ Wo^T + bo, interleaved ------
            assert phases >= 5
            p_y = tc.alloc_tile_pool(name="ysb", bufs=3, side="right")
            ps_y = tc.alloc_tile_pool(name="yps", bufs=4, space="PSUM")
            for nt in range(16):
                psy = [ps_y.tile([128, 512], FP32, tag="psy",
                                 name=f"psy{nt}_{_i}") for _i in range(2)]
                for tt in range(8):
                    for cch in range(2):
                        nc.tensor.matmul(
                            psy[cch][:],
                            attn_outT[:, tt, nt * 128:(nt + 1) * 128],
                            woT_s[:, tt, cch * 512:(cch + 1) * 512],
                            start=(tt == 0), stop=(tt == 7))
                ysb = p_y.tile([128, D], FP32, tag="ysb")
                for cch in range(2):
                    nc.vector.tensor_add(ysb[:, cch * 512:(cch + 1) * 512],
                                         psy[cch][:],
                                         bo_bc[:, cch * 512:(cch + 1) * 512])
                nc.sync.dma_start(out=y_out.ap()[nt * 128:(nt + 1) * 128, :],
                                  in_=ysb[:])
            p_y.release()
            ps_y.release()
            p_attn.release()
            p_wo.release()
            consts.release()

    nc.compile()
    return nc


def _get_nc():
    if "nc" not in _CACHE:
        _CACHE["nc"] = _build()
    return _CACHE["nc"]


def _prep_inputs(x, Wq, Wk, Wpk, bpk, Wo, bo):
    import ml_dtypes
    bf = ml_dtypes.bfloat16
    wqT = np.ascontiguousarray(Wq.T.astype(bf))
    wkT = np.ascontiguousarray(Wk.T.astype(bf))
    woT = np.ascontiguousarray(Wo.T.astype(bf))
    # Wpk (K=256, N=4096, KER=32) -> [i, k, o] contiguous
    wpkT = np.ascontiguousarray(
        Wpk.astype(bf).transpose(1, 2, 0)).reshape(N, KER * K)
    bpk2 = np.ascontiguousarray(bpk.astype(np.float32).reshape(2, 128).T)
    bo2 = np.ascontiguousarray(bo.astype(np.float32).reshape(1, D))
    ones = np.ones((128, 128), dtype=bf)
    ident = np.eye(128, dtype=bf)
    in_maps = []
    for c in range(NCORES):
        b, s = c // 2, c % 2
        xT = x[b, s * NL:(s + 1) * NL, :].T.astype(bf)  # (D, NL)
        xTim = np.ascontiguousarray(
            xT.reshape(8, 128, NL // 128, 128).transpose(2, 1, 0, 3))
        in_maps.append({
            "xTim": xTim,
            "wqT": wqT, "wkT": wkT, "woT": woT,
            "wpkT": np.ascontiguousarray(wpkT[s * NL:(s + 1) * NL]),
            "bpk": bpk2, "bo": bo2, "ones": ones, "ident": ident,
        })
    return in_maps


def kernel(x, Wq, Wk, Wpk, bpk, Wo, bo, _trace=False, _trace_kwargs=None):
    from concourse.bass_utils import run_bass_kernel_spmd

    nc = _get_nc()
    in_maps = _prep_inputs(np.asarray(x), np.asarray(Wq), np.asarray(Wk),
                           np.asarray(Wpk), np.asarray(bpk), np.asarray(Wo),
                           np.asarray(bo))
    res = run_bass_kernel_spmd(nc, in_maps, core_ids=list(range(NCORES)),
                               trace=_trace, **(_trace_kwargs or {}))
    _CACHE["last_result"] = res
    out = np.empty((B, N, D), dtype=np.float32)
    for c in range(NCORES):
        b, s = c // 2, c % 2
        out[b, s * NL:(s + 1) * NL, :] = res.results[c]["y"]
    return out
